# revision 6
# baseline (speedup 1.0000x reference)
"""RWKV time-mix (WKV) kernel for 8 Trainium2 NeuronCores — v2.

Strategy
--------
Data-parallel over B: each of the 8 cores gets 8 batches. Per core/batch,
everything runs in channel-major layout [C(part), T(free)]:

  host:   x^T (bf16), W^T (bf16), per-channel constants precomputed
  chip:   time-shift via a second (offset) DMA of the same x rows, so both
          x and x_prev views are 4B-aligned and every bf16 DVE op runs 2x.
          mixes as one STT per projection: mix = (x - xx)*tm + xx
          k/v/r projections on TensorE (bf16 -> f32 PSUM),
          WKV recurrence via DVE tensor_tensor_scan (f32 state):
              A_t = D*A_{t-1} + (e^k v)_{t-1}
              B_t = D*B_{t-1} + (e^k)_{t-1}
              num = e^u*EV + A ; den = e^u*E + B
          sigmoid folded into the denominator:
              rwkv = num / (den * (1 + e^{-r}))
                   = num * exp(-ln((1 + e^{-r}) * den))
          (exp/ln only -> single ACT table set)
          output projection on TensorE, DMA out as out^T (bf16)
  host:   concat + transpose back + f32
"""

import contextlib
import ctypes
import os
import sys
import types

import numpy as np
import ml_dtypes


def _ensure_ntff_hook():
    """The image's antenv package lacks axon_hooks; provide it (and a
    working ctypes NTFF profile hook) so trace=True paths don't crash."""
    try:
        import antenv.axon_hooks  # noqa: F401
        return
    except ImportError:
        pass
    try:
        import antenv
    except ImportError:
        antenv = types.ModuleType("antenv")
        sys.modules["antenv"] = antenv
    mod = types.ModuleType("antenv.axon_hooks")
    _hook = [None]
    mod.set_axon_ntff_profile_hook = lambda h: _hook.__setitem__(0, h)
    mod.get_axon_ntff_profile_hook = lambda: _hook[0]
    sys.modules["antenv.axon_hooks"] = mod
    sys.modules["antenv"].axon_hooks = mod

    so_path = "/opt/axon/libaxon_pjrt.so"
    if os.path.exists(so_path):
        try:
            lib = ctypes.CDLL(so_path)
            if hasattr(lib, "axon_start_nrt_profile"):
                lib.axon_start_nrt_profile.argtypes = [
                    ctypes.POINTER(ctypes.c_int64), ctypes.c_size_t]
                lib.axon_start_nrt_profile.restype = ctypes.c_int64
                lib.axon_stop_nrt_profile.argtypes = [ctypes.c_char_p]
                lib.axon_stop_nrt_profile.restype = ctypes.c_int64

                @contextlib.contextmanager
                def _profile(output_dir, device_ids):
                    import jax
                    jax.devices()
                    if device_ids:
                        ids = (ctypes.c_int64 * len(device_ids))(*device_ids)
                        rc = lib.axon_start_nrt_profile(ids, len(device_ids))
                    else:
                        rc = lib.axon_start_nrt_profile(None, 0)
                    if rc != 0:
                        raise RuntimeError(f"axon_start_nrt_profile rc={rc}")
                    try:
                        yield
                    finally:
                        n = lib.axon_stop_nrt_profile(str(output_dir).encode())
                        print(f"profile: {n} file(s) written to {output_dir}",
                              file=sys.stderr)

                mod.set_axon_ntff_profile_hook(_profile)
        except OSError:
            pass


_ensure_ntff_hook()

import concourse.bass as bass
import concourse.mybir as mybir
import concourse.tile as tile
from concourse import bacc
from concourse.bass_utils import run_bass_kernel_spmd

B, T, C = 64, 512, 1024
NCORES = 8
BPC = B // NCORES          # batches per core
P = 128
CT = C // P                # channel tiles
CTH = C // 256             # DoubleRow pair tiles
WSCALE = 32.0              # fp8 weight pre-scale for Wk/Wr

F32 = mybir.dt.float32
BF16 = mybir.dt.bfloat16
FP8 = mybir.dt.float8e4
AF = mybir.ActivationFunctionType
OP = mybir.AluOpType
PM = mybir.MatmulPerfMode

_nc_cache = {}


class _Bacc(bacc.Bacc):
    """Bacc whose ACT-table pass is pinned to the one set containing both
    exp and ln, so the Exp/Ln interleave doesn't thrash table loads."""

    def insert_act_table_loads(self):
        import concourse.mybir as mb
        from concourse.hw_specs import get_activation_tables
        from concourse.bacc import _bass_rust as br
        has_activation = any(
            isinstance(i, mb.InstActivation)
            for b in self.main_func.blocks
            for i in b.instructions
        )
        if not has_activation:
            return
        tables = []
        strip = {mb.ActivationFunctionType.Exp, mb.ActivationFunctionType.Ln}
        for name, fns in get_activation_tables(self.m.arch).items():
            if name != "natural_log_exp_and_others":
                fns = fns - strip
            tables.append((name, fns))
        br.insert_act_table_loads(self, tables)


def build_nc():
    nc = _Bacc()

    xt = nc.declare_dram_parameter("xt", [BPC, C, T], BF16, isOutput=False)
    wk = nc.declare_dram_parameter("wk", [P, CTH, 2, C], FP8, isOutput=False)
    wv = nc.declare_dram_parameter("wv", [C, C], BF16, isOutput=False)
    wr = nc.declare_dram_parameter("wr", [P, CTH, 2, C], FP8, isOutput=False)
    wo = nc.declare_dram_parameter("wo", [C, C], BF16, isOutput=False)
    # per-channel constants [P, CT, 5]: tmk, tmv, tmr, e^u, D
    cvec = nc.declare_dram_parameter("cvec", [P, CT, 5], F32, isOutput=False)
    out = nc.declare_dram_parameter("out", [BPC, C, T], BF16, isOutput=True)

    with tile.TileContext(nc) as tc:
        with (
            tc.tile_pool(name="singles", bufs=1) as singles,
            tc.tile_pool(name="xcp", bufs=2) as xcp,
            tc.tile_pool(name="xpp", bufs=2) as xpp,
            tc.tile_pool(name="mixp", bufs=2) as mixp,
            tc.tile_pool(name="difp", bufs=2) as difp,
            tc.tile_pool(name="stagec", bufs=2) as stagec,
            tc.tile_pool(name="rwkvp", bufs=2) as rwkvp,
            tc.tile_pool(name="outp", bufs=1) as outp,
            tc.tile_pool(name="ps_kvr", bufs=2, space="PSUM") as ps_kvr,
            tc.tile_pool(name="ps_out", bufs=1, space="PSUM") as ps_out,
        ):
            # ---- one-time loads ----
            cv = singles.tile([P, CT, 5], F32, tag="cvec")
            nc.sync.dma_start(out=cv[:], in_=cvec[:])

            def emit_out_pair(b, rw, dj0):
                # two output-projection groups -> one 2-bank PSUM tile,
                # one paired ACT copy, one DMA
                pso = ps_out.tile([P, 2, T], F32, tag="pso", name="pso")
                for q in range(2):
                    dj = dj0 + q
                    for kt in range(CT):
                        nc.tensor.matmul(
                            pso[:, q, :],
                            w_sb["o"][:, kt, dj * P:(dj + 1) * P],
                            rw[:, kt, :],
                            start=(kt == 0),
                            stop=(kt == CT - 1),
                        )
                osb = outp.tile([P, 2, T], BF16, tag="osb", name="osb")
                nc.scalar.copy(osb[:], pso[:])
                nc.sync.dma_start(
                    out=out[b].rearrange("(ct p) t -> p ct t", p=P)[:, dj0:dj0 + 2, :],
                    in_=osb[:],
                )

            def load_x(b):
                # xc: x[b] in [P, CT, T]; xp: the same rows shifted right by
                # one (xp[..., 0] = 0).  Two DMAs of the same DRAM region so
                # BOTH views start 4B-aligned -> bf16 DVE ops run 2x.
                xc = xcp.tile([P, CT, T], BF16, tag="xc", name="xc")
                xp = xpp.tile([P, CT, T], BF16, tag="xp", name="xp")
                nc.vector.memset(xp[:, :, 0:1], 0.0)
                src = xt[b].rearrange("(ct p) t -> p ct t", p=P)
                for ct in range(CT):
                    nc.sync.dma_start(out=xc[:, ct, :], in_=src[:, ct, :])
                    nc.sync.dma_start(out=xp[:, ct, 1:T], in_=src[:, ct, 0:T - 1])
                return xc, xp

            def make_mix_emitter(xc, xp):
                # packed [P, CT, 3(k|v|r), T] bf16; k/r also cast to fp8 (ACT)
                mixall = mixp.tile([P, CT, 3, T], BF16, tag="mixall", name="mixall")
                mk8 = mixp.tile([P, CT, T], FP8, tag="mk8", name="mk8")
                mr8 = mixp.tile([P, CT, T], FP8, tag="mr8", name="mr8")
                mix = {"k": mk8, "v": mixall[:, :, 1], "r": mr8}

                def emit_j(j):
                    # mix = (x - xx)*tm + xx : dif + per-proj scaled copy
                    # (k/r scales on ACT, v on DVE) + one broadcast add.
                    dif = difp.tile([P, T], BF16, tag="dif", name="dif")
                    nc.vector.tensor_tensor(dif[:], xc[:, j], xp[:, j], OP.subtract)
                    nc.scalar.activation(mixall[:, j, 0, :], dif[:], AF.Copy,
                                         scale=cv[:, j, 0:1])
                    nc.vector.tensor_scalar_mul(mixall[:, j, 1, :], dif[:],
                                                cv[:, j, 1:2])
                    nc.scalar.activation(mixall[:, j, 2, :], dif[:], AF.Copy,
                                         scale=cv[:, j, 2:3])
                    xp3 = xp[:, j][:, None, :].to_broadcast((P, 3, T))
                    nc.vector.tensor_tensor(mixall[:, j], mixall[:, j], xp3, OP.add)
                    if j % 2 == 1:
                        nc.scalar.copy(mk8[:, j - 1:j + 1, :],
                                       mixall[:, j - 1:j + 1, 0, :])
                        nc.scalar.copy(mr8[:, j - 1:j + 1, :],
                                       mixall[:, j - 1:j + 1, 2, :])
                return mix, emit_j

            prev = None  # (b, rwkv tile) pending output projection
            xc_cur, xp_cur = load_x(0)
            w_sb = {}
            for name, par in (("v", wv), ("o", wo)):
                t = singles.tile([P, CT, C], BF16, tag=f"w{name}", name=f"w{name}")
                src = par.rearrange("(ct p) d -> p ct d", p=P)
                for kt in range(CT):
                    nc.sync.dma_start(out=t[:, kt, :], in_=src[:, kt, :])
                w_sb[name] = t
            for name, par in (("k", wk), ("r", wr)):
                t = singles.tile([P, CTH, 2, C], FP8, tag=f"w{name}", name=f"w{name}")
                for mt in range(CTH):
                    nc.sync.dma_start(out=t[:, mt], in_=par[:, mt])
                w_sb[name] = t

            mix, emit_j0 = make_mix_emitter(xc_cur, xp_cur)
            for j in range(CT):
                emit_j0(j)

            # division stage deferred and processed per j-PAIR so the ops
            # amortize fixed overheads and bank-releasing ACT ops (E/vsb/er)
            # stay at the ACT queue front
            pending_div = [None]  # (j0, den2p, NDp, rw)

            def flush_div():
                if pending_div[0] is None:
                    return
                j0, den2p, NDp, rw2 = pending_div[0]
                pending_div[0] = None
                ld = stagec.tile([P, 2, T], F32, tag="ld", name="ld")
                nc.scalar.activation(ld[:], den2p[:], AF.Ln)
                f = stagec.tile([P, 2, T], BF16, tag="f", name="f")
                nc.scalar.activation(f[:], ld[:], AF.Exp, scale=-1.0)
                nc.vector.tensor_tensor(rw2[:, j0:j0 + 2, :], NDp[:, :, 0, :],
                                        f[:], OP.mult)

            for b in range(BPC):
                emit_mix_next = None
                if b + 1 < BPC:
                    xc_n, xp_n = load_x(b + 1)
                    mix_next, emit_mix_next = make_mix_emitter(xc_n, xp_n)
                # ---- projections + WKV per channel-tile ----
                rw = rwkvp.tile([P, CT, T], BF16, tag="rwkv", name="rwkv")
                for j in range(CT):
                    if j % 2 == 0:
                        flush_div()
                    psk = ps_kvr.tile([P, T], F32, tag="psk")
                    psv = ps_kvr.tile([P, T], F32, tag="psv")
                    psr = ps_kvr.tile([P, T], F32, tag="psr")
                    for nm, ps in (("k", psk), ("r", psr)):
                        for mt in range(CTH):
                            nc.tensor.matmul(
                                ps[:],
                                w_sb[nm][:, mt, :, j * P:(j + 1) * P],
                                mix[nm][:, 2 * mt:2 * mt + 2, :],
                                start=(mt == 0),
                                stop=(mt == CTH - 1),
                                perf_mode=PM.DoubleRow,
                            )
                    for kt in range(CT):
                        nc.tensor.matmul(
                            psv[:],
                            w_sb["v"][:, kt, j * P:(j + 1) * P],
                            mix["v"][:, kt, :],
                            start=(kt == 0),
                            stop=(kt == CT - 1),
                        )
                    # interleave prev batch's output projection into the PE
                    # stream (spreads the osb ACT copies across the batch)
                    if prev is not None and j % 2 == 1:
                        emit_out_pair(prev[0], prev[1], j - 1)

                    eu = cv[:, j, 3:4]
                    Dbc = cv[:, j, 4:5].to_broadcast((P, T - 1))

                    # EVE = [EV | E] packed so num/den is one STT
                    EVE = stagec.tile([P, 2, T], BF16, tag="EVE", name="EVE")
                    nc.scalar.activation(EVE[:, 1], psk[:], AF.Exp, scale=1.0 / WSCALE)
                    vsb = stagec.tile([P, T], BF16, tag="vsb", name="vsb")
                    nc.scalar.copy(vsb[:], psv[:])
                    if j % 2 == 0:
                        erp = stagec.tile([P, 2, T], BF16, tag="erp", name="erp")
                        NDp = stagec.tile([P, 2, 2, T], BF16, tag="NDp", name="NDp")
                    nc.scalar.activation(erp[:, j % 2], psr[:], AF.Exp,
                                         scale=-1.0 / WSCALE)

                    nc.vector.tensor_tensor(EVE[:, 0], EVE[:, 1], vsb[:], OP.mult)

                    # exclusive scans: A[:, t] = sum_{i<t} D^(t-1-i) EV_i
                    AB = stagec.tile([P, 2, T], BF16, tag="AB", name="AB")
                    nc.gpsimd.memset(AB[:, :, 0:1], 0.0)
                    nc.vector.tensor_tensor_scan(
                        AB[:, 0, 1:T], Dbc, EVE[:, 0, 0:T - 1], 0.0, OP.mult, OP.add)
                    nc.vector.tensor_tensor_scan(
                        AB[:, 1, 1:T], Dbc, EVE[:, 1, 0:T - 1], 0.0, OP.mult, OP.add)

                    # [num|den] = e^u * [EV|E] + [A|B]
                    # (TS 4x + TT 2x beats one STT, which is always 1x)
                    NDt = stagec.tile([P, 2, T], BF16, tag="NDt", name="NDt")
                    nc.vector.tensor_scalar_mul(NDt[:], EVE[:], eu)
                    nc.vector.tensor_tensor(NDp[:, j % 2], NDt[:], AB[:], OP.add)
                    if j % 2 == 1:
                        # den2 = (er + 1) * den for the whole pair
                        ert = stagec.tile([P, 2, T], BF16, tag="NDt", name="ert")
                        nc.vector.tensor_scalar_add(ert[:], erp[:], 1.0)
                        den2p = stagec.tile([P, 2, T], BF16, tag="den2p",
                                            name="den2p")
                        nc.vector.tensor_tensor(
                            den2p[:], ert[:], NDp[:, :, 1, :], OP.mult)
                        pending_div[0] = (j - 1, den2p, NDp, rw)

                    # Front-load next batch's mix: the first matmul of batch
                    # b+1 contracts over ALL its mix tiles, so they must all
                    # clear the DVE queue before the batch boundary.  Start at
                    # j=2 so the batch's own bank-releasing ACT ops (E/vsb/er
                    # of the first tiles) aren't queued behind mix-mul bursts.
                    if emit_mix_next is not None and 2 <= j < 6:
                        emit_mix_next(2 * (j - 2))
                        emit_mix_next(2 * (j - 2) + 1)

                flush_div()
                if b + 1 < BPC:
                    mix = mix_next
                    xc_cur, xp_cur = xc_n, xp_n
                prev = (b, rw)
            for dj0 in range(0, CT, 2):
                emit_out_pair(prev[0], prev[1], dj0)

    nc.compile()
    return nc


def _host_prep(x, time_decay, time_first, time_mix_k, time_mix_v, time_mix_r,
               Wk, Wv, Wr, Wo):
    bf = ml_dtypes.bfloat16
    e4 = ml_dtypes.float8_e4m3fn
    f32 = np.float32
    xt = np.ascontiguousarray(x.transpose(0, 2, 1)).astype(bf)      # [B, C, T]
    wvt = np.ascontiguousarray(np.asarray(Wv, f32).T).astype(bf)
    wot = np.ascontiguousarray(np.asarray(Wo, f32).T).astype(bf)

    def pack_fp8(W):
        # [P, CTH, 2, C]: w[p, mt, q, d] = (32*W.T)[(2*mt+q)*128 + p, d]
        Wt = (np.asarray(W, f32).T * WSCALE).astype(e4)             # [c, d]
        return np.ascontiguousarray(
            Wt.reshape(CTH, 2, P, C).transpose(2, 0, 1, 3))

    wkt = pack_fp8(Wk)
    wrt = pack_fp8(Wr)

    D = np.exp(-np.exp(np.asarray(time_decay, f32))).astype(f32)
    eu = np.exp(np.asarray(time_first, f32)).astype(f32)
    tmk = np.asarray(time_mix_k, f32).reshape(C)
    tmv = np.asarray(time_mix_v, f32).reshape(C)
    tmr = np.asarray(time_mix_r, f32).reshape(C)
    cvec = np.stack([tmk, tmv, tmr, eu, D], axis=-1)                # [C, 5]
    cvec = np.ascontiguousarray(cvec.reshape(CT, P, 5).transpose(1, 0, 2)).astype(f32)

    in_maps = []
    for i in range(NCORES):
        in_maps.append({
            "xt": xt[i * BPC:(i + 1) * BPC],
            "wk": wkt, "wv": wvt, "wr": wrt, "wo": wot,
            "cvec": cvec,
        })
    return in_maps


def kernel(x, time_decay, time_first, time_mix_k, time_mix_v, time_mix_r,
           Wk, Wv, Wr, Wo):
    x = np.asarray(x, np.float32)
    in_maps = _host_prep(x, time_decay, time_first, time_mix_k, time_mix_v,
                         time_mix_r, Wk, Wv, Wr, Wo)
    if "nc" not in _nc_cache:
        _nc_cache["nc"] = build_nc()
    res = run_bass_kernel_spmd(_nc_cache["nc"], in_maps, core_ids=list(range(NCORES)))
    _nc_cache["last_results"] = res
    full = np.concatenate([res.results[i]["out"] for i in range(NCORES)], axis=0)
    return np.ascontiguousarray(full.astype(np.float32).transpose(0, 2, 1))


# revision 7
# speedup vs baseline: 1.0024x; 1.0024x over previous
"""RWKV time-mix (WKV) kernel for 8 Trainium2 NeuronCores.

Strategy
--------
Data-parallel over B: each of the 8 cores gets 8 batches. Per core/batch,
everything runs in channel-major layout [C(part), T(free)]:

  host:   x^T (bf16), W^T (bf16), per-channel constants precomputed
  chip:   time-shift via a second (offset) DMA of the same x rows, so both
          x and x_prev views are 4B-aligned and every bf16 DVE op runs 2x.
          mix = (x - xx)*tm + xx (k/r scales on ACT, v on DVE, one
          broadcast add); k/r mixes cast to fp8 e4m3 by paired ACT copies
          k/r projections: fp8 DoubleRow matmuls (weights pre-scaled x32,
          undone via free ACT exp scale); v/out projections bf16,
          WKV recurrence via DVE tensor_tensor_scan (f32 state):
              A_t = D*A_{t-1} + (e^k v)_{t-1}
              B_t = D*B_{t-1} + (e^k)_{t-1}
              [num|den] = e^u*[EV|E] + [A|B]  (TS 4x + TT 2x, not STT)
          sigmoid folded into the denominator:
              rwkv = num / (den * (1 + e^{-r}))
                   = num * exp(-ln((1 + e^{-r}) * den))
          (exp/ln only -> single ACT table set); division processed per
          j-pair, pipelined one pair behind; next-batch mix front-loaded
          at j in [2,6) so bank-releasing ACT ops lead each batch
          output projection interleaved per pair: 2 groups -> one 2-bank
          PSUM tile, one paired ACT copy, one DMA out^T (bf16)
  host:   concat + transpose back + f32
"""

import contextlib
import ctypes
import os
import sys
import types

import numpy as np
import ml_dtypes


def _ensure_ntff_hook():
    """The image's antenv package lacks axon_hooks; provide it (and a
    working ctypes NTFF profile hook) so trace=True paths don't crash."""
    try:
        import antenv.axon_hooks  # noqa: F401
        return
    except ImportError:
        pass
    try:
        import antenv
    except ImportError:
        antenv = types.ModuleType("antenv")
        sys.modules["antenv"] = antenv
    mod = types.ModuleType("antenv.axon_hooks")
    _hook = [None]
    mod.set_axon_ntff_profile_hook = lambda h: _hook.__setitem__(0, h)
    mod.get_axon_ntff_profile_hook = lambda: _hook[0]
    sys.modules["antenv.axon_hooks"] = mod
    sys.modules["antenv"].axon_hooks = mod

    so_path = "/opt/axon/libaxon_pjrt.so"
    if os.path.exists(so_path):
        try:
            lib = ctypes.CDLL(so_path)
            if hasattr(lib, "axon_start_nrt_profile"):
                lib.axon_start_nrt_profile.argtypes = [
                    ctypes.POINTER(ctypes.c_int64), ctypes.c_size_t]
                lib.axon_start_nrt_profile.restype = ctypes.c_int64
                lib.axon_stop_nrt_profile.argtypes = [ctypes.c_char_p]
                lib.axon_stop_nrt_profile.restype = ctypes.c_int64

                @contextlib.contextmanager
                def _profile(output_dir, device_ids):
                    import jax
                    jax.devices()
                    if device_ids:
                        ids = (ctypes.c_int64 * len(device_ids))(*device_ids)
                        rc = lib.axon_start_nrt_profile(ids, len(device_ids))
                    else:
                        rc = lib.axon_start_nrt_profile(None, 0)
                    if rc != 0:
                        raise RuntimeError(f"axon_start_nrt_profile rc={rc}")
                    try:
                        yield
                    finally:
                        n = lib.axon_stop_nrt_profile(str(output_dir).encode())
                        print(f"profile: {n} file(s) written to {output_dir}",
                              file=sys.stderr)

                mod.set_axon_ntff_profile_hook(_profile)
        except OSError:
            pass


_ensure_ntff_hook()

import concourse.bass as bass
import concourse.mybir as mybir
import concourse.tile as tile
from concourse import bacc
from concourse.bass_utils import run_bass_kernel_spmd

B, T, C = 64, 512, 1024
NCORES = 8
BPC = B // NCORES          # batches per core
P = 128
CT = C // P                # channel tiles
CTH = C // 256             # DoubleRow pair tiles
WSCALE = 32.0              # fp8 weight pre-scale for Wk/Wr

F32 = mybir.dt.float32
BF16 = mybir.dt.bfloat16
FP8 = mybir.dt.float8e4
AF = mybir.ActivationFunctionType
OP = mybir.AluOpType
PM = mybir.MatmulPerfMode

_nc_cache = {}


class _Bacc(bacc.Bacc):
    """Bacc whose ACT-table pass is pinned to the one set containing both
    exp and ln, so the Exp/Ln interleave doesn't thrash table loads."""

    def insert_act_table_loads(self):
        import concourse.mybir as mb
        from concourse.hw_specs import get_activation_tables
        from concourse.bacc import _bass_rust as br
        has_activation = any(
            isinstance(i, mb.InstActivation)
            for b in self.main_func.blocks
            for i in b.instructions
        )
        if not has_activation:
            return
        tables = []
        strip = {mb.ActivationFunctionType.Exp, mb.ActivationFunctionType.Ln}
        for name, fns in get_activation_tables(self.m.arch).items():
            if name != "natural_log_exp_and_others":
                fns = fns - strip
            tables.append((name, fns))
        br.insert_act_table_loads(self, tables)


def build_nc():
    nc = _Bacc()

    xt = nc.declare_dram_parameter("xt", [BPC, C, T], BF16, isOutput=False)
    wk = nc.declare_dram_parameter("wk", [P, CTH, 2, C], FP8, isOutput=False)
    wv = nc.declare_dram_parameter("wv", [C, C], BF16, isOutput=False)
    wr = nc.declare_dram_parameter("wr", [P, CTH, 2, C], FP8, isOutput=False)
    wo = nc.declare_dram_parameter("wo", [C, C], BF16, isOutput=False)
    # per-channel constants [P, CT, 5]: tmk, tmv, tmr, e^u, D
    cvec = nc.declare_dram_parameter("cvec", [P, CT, 5], F32, isOutput=False)
    out = nc.declare_dram_parameter("out", [BPC, C, T], BF16, isOutput=True)

    with tile.TileContext(nc) as tc:
        with (
            tc.tile_pool(name="singles", bufs=1) as singles,
            tc.tile_pool(name="xcp", bufs=2) as xcp,
            tc.tile_pool(name="xpp", bufs=2) as xpp,
            tc.tile_pool(name="mixp", bufs=2) as mixp,
            tc.tile_pool(name="difp", bufs=2) as difp,
            tc.tile_pool(name="stagec", bufs=2) as stagec,
            tc.tile_pool(name="rwkvp", bufs=2) as rwkvp,
            tc.tile_pool(name="outp", bufs=1) as outp,
            tc.tile_pool(name="ps_kvr", bufs=2, space="PSUM") as ps_kvr,
            tc.tile_pool(name="ps_out", bufs=1, space="PSUM") as ps_out,
        ):
            # ---- one-time loads ----
            cv = singles.tile([P, CT, 5], F32, tag="cvec")
            nc.sync.dma_start(out=cv[:], in_=cvec[:])

            def emit_out_pair(b, rw, dj0):
                # two output-projection groups -> one 2-bank PSUM tile,
                # one paired ACT copy, one DMA
                pso = ps_out.tile([P, 2, T], F32, tag="pso", name="pso")
                for q in range(2):
                    dj = dj0 + q
                    for kt in range(CT):
                        nc.tensor.matmul(
                            pso[:, q, :],
                            w_sb["o"][:, kt, dj * P:(dj + 1) * P],
                            rw[:, kt, :],
                            start=(kt == 0),
                            stop=(kt == CT - 1),
                        )
                osb = outp.tile([P, 2, T], BF16, tag="osb", name="osb")
                nc.scalar.copy(osb[:], pso[:])
                nc.sync.dma_start(
                    out=out[b].rearrange("(ct p) t -> p ct t", p=P)[:, dj0:dj0 + 2, :],
                    in_=osb[:],
                )

            def load_x(b):
                # xc: x[b] in [P, CT, T]; xp: the same rows shifted right by
                # one (xp[..., 0] = 0).  Two DMAs of the same DRAM region so
                # BOTH views start 4B-aligned -> bf16 DVE ops run 2x.
                xc = xcp.tile([P, CT, T], BF16, tag="xc", name="xc")
                xp = xpp.tile([P, CT, T], BF16, tag="xp", name="xp")
                nc.vector.memset(xp[:, :, 0:1], 0.0)
                src = xt[b].rearrange("(ct p) t -> p ct t", p=P)
                for ct in range(CT):
                    nc.sync.dma_start(out=xc[:, ct, :], in_=src[:, ct, :])
                    nc.sync.dma_start(out=xp[:, ct, 1:T], in_=src[:, ct, 0:T - 1])
                return xc, xp

            def make_mix_emitter(xc, xp):
                # packed [P, CT, 3(k|v|r), T] bf16; k/r also cast to fp8 (ACT)
                mixall = mixp.tile([P, CT, 3, T], BF16, tag="mixall", name="mixall")
                mk8 = mixp.tile([P, CT, T], FP8, tag="mk8", name="mk8")
                mr8 = mixp.tile([P, CT, T], FP8, tag="mr8", name="mr8")
                mix = {"k": mk8, "v": mixall[:, :, 1], "r": mr8}

                def emit_j(j):
                    # mix = (x - xx)*tm + xx : dif + per-proj scaled copy
                    # (k/r scales on ACT, v on DVE) + one broadcast add.
                    dif = difp.tile([P, T], BF16, tag="dif", name="dif")
                    nc.vector.tensor_tensor(dif[:], xc[:, j], xp[:, j], OP.subtract)
                    nc.scalar.activation(mixall[:, j, 0, :], dif[:], AF.Copy,
                                         scale=cv[:, j, 0:1])
                    nc.vector.tensor_scalar_mul(mixall[:, j, 1, :], dif[:],
                                                cv[:, j, 1:2])
                    nc.scalar.activation(mixall[:, j, 2, :], dif[:], AF.Copy,
                                         scale=cv[:, j, 2:3])
                    xp3 = xp[:, j][:, None, :].to_broadcast((P, 3, T))
                    nc.vector.tensor_tensor(mixall[:, j], mixall[:, j], xp3, OP.add)
                    if j % 2 == 1:
                        nc.scalar.copy(mk8[:, j - 1:j + 1, :],
                                       mixall[:, j - 1:j + 1, 0, :])
                        nc.scalar.copy(mr8[:, j - 1:j + 1, :],
                                       mixall[:, j - 1:j + 1, 2, :])
                return mix, emit_j

            prev = None  # (b, rwkv tile) pending output projection
            xc_cur, xp_cur = load_x(0)
            w_sb = {}
            for name, par in (("v", wv), ("o", wo)):
                t = singles.tile([P, CT, C], BF16, tag=f"w{name}", name=f"w{name}")
                src = par.rearrange("(ct p) d -> p ct d", p=P)
                for kt in range(CT):
                    nc.sync.dma_start(out=t[:, kt, :], in_=src[:, kt, :])
                w_sb[name] = t
            for name, par in (("k", wk), ("r", wr)):
                t = singles.tile([P, CTH, 2, C], FP8, tag=f"w{name}", name=f"w{name}")
                for mt in range(CTH):
                    nc.sync.dma_start(out=t[:, mt], in_=par[:, mt])
                w_sb[name] = t

            mix, emit_j0 = make_mix_emitter(xc_cur, xp_cur)
            for j in range(CT):
                emit_j0(j)

            # division stage deferred and processed per j-PAIR so the ops
            # amortize fixed overheads and bank-releasing ACT ops (E/vsb/er)
            # stay at the ACT queue front
            pending_div = [None]  # (j0, den2p, NDp, rw)

            def flush_div():
                if pending_div[0] is None:
                    return
                j0, den2p, NDp, rw2 = pending_div[0]
                pending_div[0] = None
                ld = stagec.tile([P, 2, T], F32, tag="ld", name="ld")
                nc.scalar.activation(ld[:], den2p[:], AF.Ln)
                f = stagec.tile([P, 2, T], BF16, tag="f", name="f")
                nc.scalar.activation(f[:], ld[:], AF.Exp, scale=-1.0)
                nc.vector.tensor_tensor(rw2[:, j0:j0 + 2, :], NDp[:, :, 0, :],
                                        f[:], OP.mult)

            for b in range(BPC):
                emit_mix_next = None
                if b + 1 < BPC:
                    xc_n, xp_n = load_x(b + 1)
                    mix_next, emit_mix_next = make_mix_emitter(xc_n, xp_n)
                # ---- projections + WKV per channel-tile ----
                rw = rwkvp.tile([P, CT, T], BF16, tag="rwkv", name="rwkv")
                for j in range(CT):
                    if j % 2 == 0:
                        flush_div()
                    psk = ps_kvr.tile([P, T], F32, tag="psk")
                    psv = ps_kvr.tile([P, T], F32, tag="psv")
                    psr = ps_kvr.tile([P, T], F32, tag="psr")
                    for nm, ps in (("k", psk), ("r", psr)):
                        for mt in range(CTH):
                            nc.tensor.matmul(
                                ps[:],
                                w_sb[nm][:, mt, :, j * P:(j + 1) * P],
                                mix[nm][:, 2 * mt:2 * mt + 2, :],
                                start=(mt == 0),
                                stop=(mt == CTH - 1),
                                perf_mode=PM.DoubleRow,
                            )
                    for kt in range(CT):
                        nc.tensor.matmul(
                            psv[:],
                            w_sb["v"][:, kt, j * P:(j + 1) * P],
                            mix["v"][:, kt, :],
                            start=(kt == 0),
                            stop=(kt == CT - 1),
                        )
                    # interleave prev batch's output projection into the PE
                    # stream (spreads the osb ACT copies across the batch)
                    if prev is not None and j % 2 == 1:
                        emit_out_pair(prev[0], prev[1], j - 1)

                    eu = cv[:, j, 3:4]
                    Dbc = cv[:, j, 4:5].to_broadcast((P, T - 1))

                    # EVE = [EV | E] packed so num/den is one STT
                    EVE = stagec.tile([P, 2, T], BF16, tag="EVE", name="EVE")
                    nc.scalar.activation(EVE[:, 1], psk[:], AF.Exp, scale=1.0 / WSCALE)
                    vsb = stagec.tile([P, T], BF16, tag="vsb", name="vsb")
                    nc.scalar.copy(vsb[:], psv[:])
                    if j % 2 == 0:
                        erp = stagec.tile([P, 2, T], BF16, tag="erp", name="erp")
                        NDp = stagec.tile([P, 2, 2, T], BF16, tag="NDp", name="NDp")
                    nc.scalar.activation(erp[:, j % 2], psr[:], AF.Exp,
                                         scale=-1.0 / WSCALE)

                    nc.vector.tensor_tensor(EVE[:, 0], EVE[:, 1], vsb[:], OP.mult)

                    # exclusive scans: A[:, t] = sum_{i<t} D^(t-1-i) EV_i
                    AB = stagec.tile([P, 2, T], BF16, tag="AB", name="AB")
                    nc.gpsimd.memset(AB[:, :, 0:1], 0.0)
                    nc.vector.tensor_tensor_scan(
                        AB[:, 0, 1:T], Dbc, EVE[:, 0, 0:T - 1], 0.0, OP.mult, OP.add)
                    nc.vector.tensor_tensor_scan(
                        AB[:, 1, 1:T], Dbc, EVE[:, 1, 0:T - 1], 0.0, OP.mult, OP.add)

                    # [num|den] = e^u * [EV|E] + [A|B]
                    # (TS 4x + TT 2x beats one STT, which is always 1x)
                    NDt = stagec.tile([P, 2, T], BF16, tag="NDt", name="NDt")
                    nc.vector.tensor_scalar_mul(NDt[:], EVE[:], eu)
                    nc.vector.tensor_tensor(NDp[:, j % 2], NDt[:], AB[:], OP.add)
                    if j % 2 == 1:
                        # den2 = (er + 1) * den for the whole pair
                        ert = stagec.tile([P, 2, T], BF16, tag="NDt", name="ert")
                        nc.vector.tensor_scalar_add(ert[:], erp[:], 1.0)
                        den2p = stagec.tile([P, 2, T], BF16, tag="den2p",
                                            name="den2p")
                        nc.vector.tensor_tensor(
                            den2p[:], ert[:], NDp[:, :, 1, :], OP.mult)
                        pending_div[0] = (j - 1, den2p, NDp, rw)

                    # Front-load next batch's mix: the first matmul of batch
                    # b+1 contracts over ALL its mix tiles, so they must all
                    # clear the DVE queue before the batch boundary.  Start at
                    # j=2 so the batch's own bank-releasing ACT ops (E/vsb/er
                    # of the first tiles) aren't queued behind mix-mul bursts.
                    if emit_mix_next is not None and 2 <= j < 6:
                        emit_mix_next(2 * (j - 2))
                        emit_mix_next(2 * (j - 2) + 1)

                flush_div()
                if b + 1 < BPC:
                    mix = mix_next
                    xc_cur, xp_cur = xc_n, xp_n
                prev = (b, rw)
            for dj0 in range(0, CT, 2):
                emit_out_pair(prev[0], prev[1], dj0)

    nc.compile()
    return nc


def _host_prep(x, time_decay, time_first, time_mix_k, time_mix_v, time_mix_r,
               Wk, Wv, Wr, Wo):
    bf = ml_dtypes.bfloat16
    e4 = ml_dtypes.float8_e4m3fn
    f32 = np.float32
    xt = np.ascontiguousarray(x.transpose(0, 2, 1)).astype(bf)      # [B, C, T]
    wvt = np.ascontiguousarray(np.asarray(Wv, f32).T).astype(bf)
    wot = np.ascontiguousarray(np.asarray(Wo, f32).T).astype(bf)

    def pack_fp8(W):
        # [P, CTH, 2, C]: w[p, mt, q, d] = (32*W.T)[(2*mt+q)*128 + p, d]
        Wt = (np.asarray(W, f32).T * WSCALE).astype(e4)             # [c, d]
        return np.ascontiguousarray(
            Wt.reshape(CTH, 2, P, C).transpose(2, 0, 1, 3))

    wkt = pack_fp8(Wk)
    wrt = pack_fp8(Wr)

    D = np.exp(-np.exp(np.asarray(time_decay, f32))).astype(f32)
    eu = np.exp(np.asarray(time_first, f32)).astype(f32)
    tmk = np.asarray(time_mix_k, f32).reshape(C)
    tmv = np.asarray(time_mix_v, f32).reshape(C)
    tmr = np.asarray(time_mix_r, f32).reshape(C)
    cvec = np.stack([tmk, tmv, tmr, eu, D], axis=-1)                # [C, 5]
    cvec = np.ascontiguousarray(cvec.reshape(CT, P, 5).transpose(1, 0, 2)).astype(f32)

    in_maps = []
    for i in range(NCORES):
        in_maps.append({
            "xt": xt[i * BPC:(i + 1) * BPC],
            "wk": wkt, "wv": wvt, "wr": wrt, "wo": wot,
            "cvec": cvec,
        })
    return in_maps


def kernel(x, time_decay, time_first, time_mix_k, time_mix_v, time_mix_r,
           Wk, Wv, Wr, Wo):
    x = np.asarray(x, np.float32)
    in_maps = _host_prep(x, time_decay, time_first, time_mix_k, time_mix_v,
                         time_mix_r, Wk, Wv, Wr, Wo)
    if "nc" not in _nc_cache:
        _nc_cache["nc"] = build_nc()
    res = run_bass_kernel_spmd(_nc_cache["nc"], in_maps, core_ids=list(range(NCORES)))
    _nc_cache["last_results"] = res
    full = np.concatenate([res.results[i]["out"] for i in range(NCORES)], axis=0)
    return np.ascontiguousarray(full.astype(np.float32).transpose(0, 2, 1))
